# revision 63
# baseline (speedup 1.0000x reference)
"""MultiHeadAttention (qk-LayerNorm + RoPE) Trainium2 kernel, 8 NeuronCores.

Sharding: batch (4) x query-token-half (2x512 rows).  Core c handles batch
c//2 and output rows [512*(c%2), 512*(c%2)+512).  Each core computes K/V
projections for ALL 1024 tokens of its batch (duplicated within the pair --
cheaper than any collective: the baseline's ReduceScatter + device barrier
cost ~170us) and the Q projection for its 512 rows, so every core emits 512
complete output rows with ZERO cross-device communication.

Tricks:
- LN mean-centering is folded into Wq/Wk on the host (subtract per-head
  column means), so projections come out exactly mean-centered and only the
  variance is computed on-chip (ACT square + DVE reduce).
- rope(LN(x)) = inv * (x*A + rot(x)*B) since inv/shift are constant along d
  and shift == 0 -- one fused normalize+rope pass on DVE.
- D^-0.5 is folded into the q-side inv via bias = 64*eps, scale = 1 in the
  sqrt (giving 8*std directly).
- Attention per head-pair: scores for both heads land in one [128,1024]
  psum tile -> single 1024-wide exp.  V carries an appended ones column so
  ctx_unnorm and sum-of-exp come from one accumulated matmul; the odd head
  uses a 128-wide zero-padded V block so its ctx lands on psum partitions
  64:128 (sumexp on row 63), which keeps the o_proj contraction head-paired
  at K=128.
- All matmuls bf16 (PE upconverts to FP22 internally); psum f32.
"""
import sys

for _p in ("/opt/trn_rl_repo", "/root/.axon_site", "/root/.axon_site/_ro/trn_rl_repo",
           "/root/.axon_site/_ro/pypackages"):
    if _p not in sys.path:
        sys.path.append(_p)

import numpy as np
import ml_dtypes

import concourse.bass as bass
import concourse.tile as tile
from concourse import bacc, mybir
from concourse.bass_utils import run_bass_kernel_spmd
from concourse.masks import make_identity

F32 = mybir.dt.float32
F32R = mybir.dt.float32r
BF16 = mybir.dt.bfloat16
BF = ml_dtypes.bfloat16
P = 128
B, L, C, H, D = 4, 1024, 1024, 16, 64
NT = L // P        # 8 k/v token tiles
NQ = 4             # q token tiles (512 rows per core)
NCK = C // P       # 8 contraction chunks
NHP = H // 2       # 8 head pairs
QL = 512           # query rows per core
VW = 256           # v slot width per head pair: 128 (even) + 128 (odd)
THETA = 50000.0
EPS = 1e-5

_NC_CACHE = {}


def _build_nc():
    nc = bacc.Bacc("TRN2", target_bir_lowering=False, debug=False, num_devices=8)

    xT_d = nc.dram_tensor("xT", [C, L], BF16, kind="ExternalInput")
    wq_d = nc.dram_tensor("wq", [C, C], BF16, kind="ExternalInput")
    wk_d = nc.dram_tensor("wk", [C, C], BF16, kind="ExternalInput")
    wv_d = nc.dram_tensor("wv", [C, C], BF16, kind="ExternalInput")
    wo_d = nc.dram_tensor("wo", [NHP, P, C], BF16, kind="ExternalInput")
    aq_d = nc.dram_tensor("aq", [QL, D], F32, kind="ExternalInput")
    bq_d = nc.dram_tensor("bq", [QL, D], F32, kind="ExternalInput")
    ak_d = nc.dram_tensor("ak", [L, D], F32, kind="ExternalInput")
    bk_d = nc.dram_tensor("bk", [L, D], F32, kind="ExternalInput")
    out_d = nc.dram_tensor("out", [QL, C], F32, kind="ExternalOutput")

    with tile.TileContext(nc) as tc:
        with (
            tc.tile_pool(name="const", bufs=1) as constp,
            tc.tile_pool(name="w", bufs=1) as wpool,
            tc.tile_pool(name="big", bufs=1) as bigp,
            tc.tile_pool(name="sq", bufs=2) as sqp,
            tc.tile_pool(name="nrm", bufs=2) as nrmp,
            tc.tile_pool(name="st", bufs=2) as stp,
            tc.tile_pool(name="exp", bufs=4) as expp,
            tc.tile_pool(name="rb", bufs=2) as rbp,
            tc.tile_pool(name="rc", bufs=2) as rcp,
            tc.tile_pool(name="fin", bufs=2) as finp,
        ):
            ident = constp.tile([P, P], BF16)
            make_identity(nc, ident)
            eps_k = constp.tile([P, 1], F32)
            nc.vector.memset(eps_k[:], EPS)
            eps_q = constp.tile([P, 1], F32)
            nc.vector.memset(eps_q[:], float(D) * EPS)
            # per-ck chunks so matmuls start as soon as the first chunks land;
            # x arrives in 256-token column blocks so k-tile 0 is gated by
            # only wk + first x block (~2.5MB) instead of the full 6MB.
            def wchunks(d_, nm):
                lst = []
                for ck in range(NCK):
                    t_ = wpool.tile([P, C], BF16, tag=f"{nm}{ck}", name=f"{nm}{ck}")
                    nc.sync.dma_start(
                        t_[:], d_.ap().rearrange("(k p) o -> p k o", p=P)[:, ck, :])
                    lst.append(t_)
                return lst

            xr = xT_d.ap().rearrange("(k p) t -> p k t", p=P)
            xt_l = [[wpool.tile([P, 256], BF16, tag=f"xt{ck}_{tb}",
                                name=f"xt{ck}_{tb}") for tb in range(4)]
                    for ck in range(NCK)]

            def xchunks(tb):
                for ck in range(NCK):
                    nc.sync.dma_start(xt_l[ck][tb][:],
                                      xr[:, ck, tb * 256:(tb + 1) * 256])

            # critical prefix: interleave wk[ck] with x[ck] tb0 so the first
            # psk accumulation chain starts after one pair (~0.5MB) instead
            # of the whole wk + x block
            wk_l = []
            wkr = wk_d.ap().rearrange("(k p) o -> p k o", p=P)
            for ck in range(NCK):
                t_ = wpool.tile([P, C], BF16, tag=f"wk{ck}", name=f"wk{ck}")
                nc.sync.dma_start(t_[:], wkr[:, ck, :])
                wk_l.append(t_)
                nc.sync.dma_start(xt_l[ck][0][:], xr[:, ck, 0:256])
            wv_l = wchunks(wv_d, "wv")
            # rope tables are first needed by the tile-0 DVE chain (~8us in)
            ak_t = constp.tile([P, NT, D], F32)
            nc.sync.dma_start(ak_t[:], ak_d.ap().rearrange("(t p) d -> p t d", p=P))
            bk_t = constp.tile([P, NT, D], F32)
            nc.sync.dma_start(bk_t[:], bk_d.ap().rearrange("(t p) d -> p t d", p=P))
            aq_t = constp.tile([P, NQ, D], F32)
            nc.sync.dma_start(aq_t[:], aq_d.ap().rearrange("(t p) d -> p t d", p=P))
            bq_t = constp.tile([P, NQ, D], F32)
            nc.sync.dma_start(bq_t[:], bq_d.ap().rearrange("(t p) d -> p t d", p=P))
            xchunks(1)
            wq_l = wchunks(wq_d, "wq")
            xchunks(2)
            xchunks(3)
            wo_t = wpool.tile([P, NHP, C], BF16, name="wo")
            nc.sync.dma_start(wo_t[:], wo_d.ap().rearrange("g p o -> p g o"))

            def xsl(ck, tok):
                return xt_l[ck][tok // 256][:, tok % 256:tok % 256 + P]

            # v with 128-wide per-head blocks, zero padded so every psum row
            # the epilogue reads is written.  Even head: v at 0:64, ones col
            # at 64 (sumexp -> psum row 64, ctx rows 0:64).  Odd head: ones
            # col at 32 (sumexp -> row 32), v at 64:128 (ctx rows 64:128).
            v_sb = bigp.tile([P, NT, NHP, VW], BF16)
            nc.gpsimd.memset(v_sb[:, :, :, 65:192], 0.0)
            nc.gpsimd.memset(v_sb[:, :, :, 64:65], 1.0)
            nc.gpsimd.memset(v_sb[:, :, :, 160:161], 1.0)

            kT = bigp.tile([P, NHP, L], BF16)
            qT = bigp.tile([P, NHP, QL], BF16)
            ctxT = bigp.tile([P, NHP, QL], BF16)

            # ---------------- Phase 1: QKV + LN + RoPE + transpose ----------
            units = [("k", i) for i in range(NT)] + [("q", i) for i in range(NQ)]
            with tc.tile_pool(name="ps1", bufs=2, space="PSUM") as ps1, \
                 tc.tile_pool(name="pst", bufs=2, space="PSUM") as pst:
                for kind, ti in units:
                    is_k = kind == "k"
                    tok = ti * P
                    w_l = wk_l if is_k else wq_l
                    a_t, b_t = (ak_t, bk_t) if is_k else (aq_t, bq_t)
                    ps = ps1.tile([P, C], F32, tag="ps")
                    for m in range(2):
                        for ck in range(NCK):
                            nc.tensor.matmul(ps[:, bass.ts(m, 512)],
                                             xsl(ck, tok),
                                             w_l[ck][:, bass.ts(m, 512)],
                                             start=(ck == 0), stop=(ck == NCK - 1))
                    if is_k:
                        psv = ps1.tile([P, C], F32, tag="psv", bufs=1)
                        for m in range(2):
                            for ck in range(NCK):
                                nc.tensor.matmul(psv[:, bass.ts(m, 512)],
                                                 xsl(ck, tok),
                                                 wv_l[ck][:, bass.ts(m, 512)],
                                                 start=(ck == 0), stop=(ck == NCK - 1))
                    psr = ps[:].rearrange("p (h d) -> p h d", d=D)

                    # variance (mean is exactly 0: weights are pre-centered)
                    sq = sqp.tile([P, C], BF16, tag="sq")
                    nc.scalar.square(sq[:], ps[:])
                    vs = stp.tile([P, H], BF16, tag="vs")
                    with nc.allow_low_precision(reason="bf16 var accum, 0.4% rel"):
                        nc.vector.reduce_sum(
                            vs[:], sq[:].rearrange("p (h d) -> p h d", d=D),
                            axis=mybir.AxisListType.X)
                    std = stp.tile([P, H], F32, tag="std")
                    # k: sqrt(sum/64 + eps) = std ; q: sqrt(sum + 64eps) = 8*std
                    nc.scalar.activation(std[:], vs[:],
                                         mybir.ActivationFunctionType.Sqrt,
                                         bias=(eps_k[:] if is_k else eps_q[:]),
                                         scale=(1.0 / D if is_k else 1.0))
                    inv = stp.tile([P, H], F32, tag="inv")
                    nc.vector.reciprocal(inv[:], std[:])

                    # rope on raw centered ps, then scale by inv
                    nrm = nrmp.tile([P, H, D], BF16, tag="nrm")
                    tmp = nrmp.tile([P, H, D], BF16, tag="tmp", bufs=1)
                    a_b = a_t[:, ti, :].rearrange("p d -> p () d").to_broadcast((P, H, D))
                    nc.vector.tensor_mul(nrm[:], psr, a_b)
                    h_ = D // 2
                    nc.vector.tensor_mul(
                        tmp[:, :, 0:h_], psr[:, :, h_:D],
                        b_t[:, ti, 0:h_].rearrange("p d -> p () d").to_broadcast((P, H, h_)))
                    nc.vector.tensor_mul(
                        tmp[:, :, h_:D], psr[:, :, 0:h_],
                        b_t[:, ti, h_:D].rearrange("p d -> p () d").to_broadcast((P, H, h_)))
                    nc.vector.tensor_add(nrm[:], nrm[:], tmp[:])
                    inv_b = inv[:].rearrange("p h -> p h ()").to_broadcast((P, H, D))
                    nc.vector.tensor_mul(nrm[:], nrm[:], inv_b)

                    # transpose to [pair-channel, token]
                    dst = kT if is_k else qT
                    for g in range(2):
                        pt = pst.tile([P, 512], BF16, tag="pt")
                        for q4 in range(4):
                            hp = 4 * g + q4
                            nc.tensor.transpose(
                                pt[:, q4 * P:(q4 + 1) * P],
                                nrm[:, 2 * hp:2 * hp + 2, :].rearrange("p h d -> p (h d)"),
                                ident[:])
                        nc.scalar.copy(
                            dst[:, 4 * g:4 * g + 4, tok:tok + P],
                            pt[:].rearrange("p (f t) -> p f t", t=P))
                    if is_k:
                        psvr = psv[:].rearrange("p (g two d) -> p g two d", two=2, d=D)
                        nc.scalar.copy(v_sb[:, ti, :, 0:D], psvr[:, :, 0, :])
                        nc.scalar.copy(v_sb[:, ti, :, 192:VW], psvr[:, :, 1, :])

            # ---------------- Phase 2: attention per head pair --------------
            # The epilogue (recip -> rb broadcast -> normalize) is pipelined
            # one head-pair behind so the reciprocal chain never stalls the
            # in-order PE queue.  The reciprocal itself is made partition-
            # parallel: the 32-row psum block holding the sumexp row is
            # block-transposed (DVE 32x32 stream transpose) so the 512
            # values spread over 32 lanes (16/lane, 8 iter-div cycles each),
            # then transposed back into row form for the rb broadcast.
            sT = rbp.tile([P, 2 * QL], F32, name="sT", bufs=1)
            rcT = rbp.tile([P, 2 * QL], F32, name="rcT", bufs=1)
            rc2 = rbp.tile([P, 2 * QL], F32, name="rc2", bufs=1)
            # initialize the junk lanes once so the back-transpose reads
            # finite values
            nc.vector.memset(rcT[:], 1.0)

            with tc.tile_pool(name="pss", bufs=2, space="PSUM") as pssp, \
                 tc.tile_pool(name="psc", bufs=2, space="PSUM") as pscp, \
                 tc.tile_pool(name="rdram", bufs=2, space="DRAM") as rdramp:

                def epilogue(hp, psc):
                    # even head: sumexp row 64 cols 0:QL; odd: row 32, QL:2QL
                    for base, cl in ((64, 0), (32, QL)):
                        blk = slice(base, base + 32)
                        cs = slice(cl, cl + QL)
                        nc.vector.transpose(sT[blk, cs], psc[blk, cs])
                        nc.vector.reciprocal(rcT[blk, cl:cl + QL:32],
                                             sT[blk, cl:cl + QL:32])
                        nc.vector.transpose(rc2[blk, cs], rcT[blk, cs])
                    # broadcast the recip rows down the partitions with a
                    # DRAM-bounce DMA (partition-broadcast needs a DRAM src);
                    # keeps the whole normalize chain off the PE stream
                    rbd = rdramp.tile([2, QL], F32, tag="rbd")
                    nc.sync.dma_start(rbd[0:1, :], rc2[64:65, 0:QL])
                    nc.sync.dma_start(rbd[1:2, :], rc2[32:33, QL:2 * QL])
                    rb = rbp.tile([P, 2 * QL], F32, tag="rb")
                    nc.sync.dma_start(
                        rb[0:64, 0:QL],
                        rbd[0:1, :].to_broadcast((64, QL)))
                    nc.sync.dma_start(
                        rb[64:128, QL:2 * QL],
                        rbd[1:2, :].to_broadcast((64, QL)))
                    nc.vector.tensor_mul(ctxT[0:64, hp, :], psc[0:64, 0:QL],
                                         rb[0:64, 0:QL])
                    nc.vector.tensor_mul(ctxT[64:128, hp, :],
                                         psc[64:128, QL:2 * QL],
                                         rb[64:128, QL:2 * QL])

                def ctx_mm(psc, hp, j, ex):
                    nc.tensor.matmul(psc[:, 0:QL], v_sb[:, j, hp, 0:P],
                                     ex[:, 0:QL],
                                     start=(j == 0), stop=(j == NT - 1))
                    nc.tensor.matmul(psc[:, QL:2 * QL], v_sb[:, j, hp, P:VW],
                                     ex[:, QL:2 * QL],
                                     start=(j == 0), stop=(j == NT - 1))

                for hp in range(NHP):
                    psc = pscp.tile([P, 2 * QL], F32, tag="psc")
                    prev = None
                    for j in range(NT):
                        pss = pssp.tile([P, 2 * QL], F32, tag="pss")
                        jt = j * P
                        nc.tensor.matmul(pss[:, 0:QL],
                                         kT[0:64, hp, jt:jt + P], qT[0:64, hp, :],
                                         start=True, stop=True)
                        nc.tensor.matmul(pss[:, QL:2 * QL],
                                         kT[64:128, hp, jt:jt + P], qT[64:128, hp, :],
                                         start=True, stop=True)
                        ex = expp.tile([P, 2 * QL], BF16, tag="ex")
                        nc.scalar.activation(ex[:], pss[:],
                                             mybir.ActivationFunctionType.Exp)
                        # ctx for j-1 is emitted after scores(j): the PE never
                        # sits behind a ctx matmul that waits on exp(j)
                        if prev is not None:
                            ctx_mm(psc, hp, j - 1, prev)
                        prev = ex
                    ctx_mm(psc, hp, NT - 1, prev)
                    epilogue(hp, psc)

            # ---------------- Phase 3: output projection --------------------
            with tc.tile_pool(name="pso", bufs=2, space="PSUM") as psop:
                for qt in range(NQ):
                    pso = psop.tile([P, C], F32, tag="pso")
                    for m in range(2):
                        for g in range(NHP):
                            nc.tensor.matmul(pso[:, bass.ts(m, 512)],
                                             ctxT[:, g, qt * P:(qt + 1) * P],
                                             wo_t[:, g, bass.ts(m, 512)],
                                             start=(g == 0), stop=(g == NHP - 1))
                    osb = finp.tile([P, C], F32, tag="osb")
                    # evacuate and ship in half-tile chunks so the final DMA
                    # starts earlier
                    nc.vector.tensor_copy(osb[0:64, :], pso[0:64, :])
                    nc.sync.dma_start(out_d.ap()[qt * P:qt * P + 64, :],
                                      osb[0:64, :])
                    nc.vector.tensor_copy(osb[64:128, :], pso[64:128, :])
                    nc.sync.dma_start(out_d.ap()[qt * P + 64:(qt + 1) * P, :],
                                      osb[64:128, :])

    nc.compile()
    return nc


def _rope_tables(w, b, n_tok):
    """A[t,d], B[t,d] with the rotate-half sign folded into B."""
    inv_freq = 1.0 / THETA ** (np.arange(0, D, 2, dtype=np.float64) / D)
    freqs = np.arange(n_tok, dtype=np.float64)[:, None] * inv_freq[None, :]
    freqs = np.concatenate([freqs, freqs], axis=1)           # [n_tok, D]
    cos, sin = np.cos(freqs), np.sin(freqs)
    w = w.astype(np.float64)
    w_rot = np.concatenate([w[D // 2:], w[:D // 2]])
    sgn = np.concatenate([-np.ones(D // 2), np.ones(D // 2)])
    A = (cos * w[None, :]).astype(np.float32)
    Bt = (sin * w_rot[None, :] * sgn[None, :]).astype(np.float32)
    if np.any(b != 0):
        raise NotImplementedError("nonzero qk-norm bias not supported")
    return A, Bt


def _center_heads(W):
    """Subtract per-head mean over output rows: W[o, c] - mean_{o' in head}."""
    W = W.reshape(H, D, C)
    return (W - W.mean(axis=1, keepdims=True)).reshape(C, C)


def kernel(**inputs):
    x = np.asarray(inputs["q"], dtype=np.float32)
    Wq = np.asarray(inputs["Wq"], dtype=np.float32)
    Wk = np.asarray(inputs["Wk"], dtype=np.float32)
    Wv = np.asarray(inputs["Wv"], dtype=np.float32)
    Wo = np.asarray(inputs["Wo"], dtype=np.float32)
    bo = np.asarray(inputs["bo"], dtype=np.float32)
    assert not np.any(bo != 0), "nonzero output bias not supported"

    Ak, Bk = _rope_tables(np.asarray(inputs["kn_w"], np.float32),
                          np.asarray(inputs["kn_b"], np.float32), L)
    Aq_full, Bq_full = _rope_tables(np.asarray(inputs["qn_w"], np.float32),
                                    np.asarray(inputs["qn_b"], np.float32), L)

    wq_bf = np.ascontiguousarray(_center_heads(Wq).T).astype(BF)   # [c, o]
    wk_bf = np.ascontiguousarray(_center_heads(Wk).T).astype(BF)
    wv_bf = np.ascontiguousarray(Wv.T).astype(BF)
    wo_bf = np.ascontiguousarray(Wo.T.reshape(NHP, P, C)).astype(BF)

    if "nc" not in _NC_CACHE:
        _NC_CACHE["nc"] = _build_nc()
    nc = _NC_CACHE["nc"]

    # Each core sees its query half at x columns 0:512: roll the token axis
    # by -q0 (attention is order-invariant over keys; the k-side rope tables
    # are rolled identically so keys keep their true positions).
    in_maps = []
    for c in range(8):
        b_, half = c // 2, c % 2
        q0 = half * QL
        xTb = np.ascontiguousarray(np.roll(x[b_].T, -q0, axis=1)).astype(BF)
        in_maps.append({
            "xT": xTb,
            "wq": wq_bf, "wk": wk_bf, "wv": wv_bf, "wo": wo_bf,
            "aq": np.ascontiguousarray(Aq_full[q0:q0 + QL]),
            "bq": np.ascontiguousarray(Bq_full[q0:q0 + QL]),
            "ak": np.ascontiguousarray(np.roll(Ak, -q0, axis=0)),
            "bk": np.ascontiguousarray(np.roll(Bk, -q0, axis=0)),
        })

    res = run_bass_kernel_spmd(nc, in_maps, core_ids=list(range(8)))
    out = np.empty((B, L, C), dtype=np.float32)
    for c in range(8):
        b_, half = c // 2, c % 2
        out[b_, half * QL:(half + 1) * QL, :] = res.results[c]["out"]
    return out
